# revision 1
# baseline (speedup 1.0000x reference)
"""Single-head attention (B=8, N=2048, D=512, fp32) on 8 TRN2 NeuronCores.

Sharding: data-parallel over batch — core i computes batch element i
end-to-end (weights replicated). Per-core pipeline, all matmuls in
float32r (full-rate PE, ~1e-4 relative rounding):

  x [2048,512] --PE transpose--> xT [512,2048]   (D on partitions)
  QT = Wq^T-contract -> [512,2048],  KT likewise (D on partitions)
  V  = x @ Wv -> [2048,512]          (seq on partitions)
  per 512-wide q strip:
     S^T tile [k=128,q=512] = KT-chunk^T @ QT     (accum over D chunks)
     E = exp(S^T / sqrt(D))                        (ACT, fused scale)
     colsums  += ones[128,128]^T @ E               (PSUM accum over k tiles)
     OT[c]    += V-chunk^T @ E                     (PSUM accum over k tiles)
     OT *= 1/colsums ; PE-transpose OT -> O rows ; DMA out
"""

import numpy as np

import concourse.bass as bass
import concourse.tile as tile
from concourse import bacc, mybir
from concourse import bass_utils
from concourse.bass import ts
from concourse.masks import make_identity
from contextlib import ExitStack

B, N, D = 8, 2048, 512
P = 128
NT = N // P      # 16 seq tiles
DC = D // P      # 4 d chunks
QS = 512         # q-strip width (one PSUM bank of fp32)
NS = N // QS     # 4 strips
SOFTMAX_SCALE = 1.0 / float(np.sqrt(D))

F32 = mybir.dt.float32
F32R = mybir.dt.float32r
AF = mybir.ActivationFunctionType


def _build():
    nc = bacc.Bacc("TRN2", target_bir_lowering=False, debug=False)

    x = nc.dram_tensor("x", [N, D], F32, kind="ExternalInput").ap()
    wq = nc.dram_tensor("wq", [D, D], F32, kind="ExternalInput").ap()
    bq = nc.dram_tensor("bq", [D], F32, kind="ExternalInput").ap()
    wk = nc.dram_tensor("wk", [D, D], F32, kind="ExternalInput").ap()
    bk = nc.dram_tensor("bk", [D], F32, kind="ExternalInput").ap()
    wv = nc.dram_tensor("wv", [D, D], F32, kind="ExternalInput").ap()
    bv = nc.dram_tensor("bv", [D], F32, kind="ExternalInput").ap()
    out = nc.dram_tensor("out", [N, D], F32, kind="ExternalOutput").ap()

    with ExitStack() as ctx:
        tc = ctx.enter_context(tile.TileContext(nc))

        const = ctx.enter_context(tc.tile_pool(name="const", bufs=1))
        io512 = ctx.enter_context(tc.tile_pool(name="io512", bufs=3))
        wpool = ctx.enter_context(tc.tile_pool(name="wpool", bufs=2))
        wstage = ctx.enter_context(tc.tile_pool(name="wstage", bufs=2))
        big = ctx.enter_context(tc.tile_pool(name="big", bufs=1))
        epool = ctx.enter_context(tc.tile_pool(name="epool", bufs=3))
        otpool = ctx.enter_context(tc.tile_pool(name="otpool", bufs=2))
        rpool = ctx.enter_context(tc.tile_pool(name="rpool", bufs=2))

        # constants
        ident = const.tile([P, P], F32)
        make_identity(nc, ident)
        ones_f = const.tile([P, P], F32)
        nc.vector.memset(ones_f, 1.0)
        ones_r = const.tile([P, P], F32R)
        nc.vector.tensor_copy(out=ones_r[:], in_=ones_f[:])

        # biases: per-partition layout [P, DC] for QT/KT (d on partitions)
        bq_sb = const.tile([P, DC], F32)
        nc.sync.dma_start(bq_sb[:], bq.rearrange("(c p) -> p c", p=P))
        bk_sb = const.tile([P, DC], F32)
        nc.sync.dma_start(bk_sb[:], bk.rearrange("(c p) -> p c", p=P))
        # bv replicated across partitions [P, D] for V tiles (d on free dim)
        bv_rep = const.tile([P, D], F32)
        nc.sync.dma_start(bv_rep[:], bv[None, :].to_broadcast((P, D)))

        # weights: [ki, ko, dout] fp32r (round via DVE copy)
        w_sb = {}
        for name, wap in (("q", wq), ("k", wk), ("v", wv)):
            wst = wstage.tile([P, DC, D], F32, tag="wstage")
            nc.sync.dma_start(wst[:], wap.rearrange("(ko ki) d -> ki ko d", ki=P))
            wr = wpool.tile([P, DC, D], F32R, tag="w")
            nc.vector.tensor_copy(out=wr[:], in_=wst[:])
            w_sb[name] = wr

        # big persistent tensors
        xT = big.tile([P, DC, N], F32R)    # x^T: d on partitions
        QT = big.tile([P, DC, N], F32R)
        KT = big.tile([P, DC, N], F32R)
        V = big.tile([P, NT, D], F32R)     # natural: seq on partitions

        # ---- phase 1: load x tiles + PE-transpose into xT ----
        with tc.tile_pool(name="ps_tr", bufs=2, space="PSUM") as ps_tr, \
             tc.tile_pool(name="ps_proj", bufs=3, space="PSUM") as ps_proj:
            for t in range(NT):
                x_t = io512.tile([P, D], F32, tag="io512")
                nc.sync.dma_start(x_t[:], x[ts(t, P), :])
                for c in range(DC):
                    tp = ps_tr.tile([P, P], F32, tag="tr")
                    nc.tensor.transpose(tp[:], x_t[:, ts(c, P)], ident)
                    nc.vector.tensor_copy(out=xT[:, c, ts(t, P)], in_=tp[:])

            # ---- phase 2: projections ----
            # QT/KT: [dout-chunk co on partitions, q on free]
            for name, dst, b_sb in (("q", QT, bq_sb), ("k", KT, bk_sb)):
                wr = w_sb[name]
                for co in range(DC):
                    for s in range(NS):
                        pq = ps_proj.tile([P, QS], F32, tag="proj")
                        for ki in range(DC):
                            nc.tensor.matmul(
                                pq[:], wr[:, ki, ts(co, P)], xT[:, ki, ts(s, QS)],
                                start=(ki == 0), stop=(ki == DC - 1),
                            )
                        # bias add (per-partition) + round to fp32r on ACT
                        nc.scalar.activation(
                            dst[:, co, ts(s, QS)], pq[:], AF.Identity,
                            bias=b_sb[:, co:co + 1],
                        )
            # V: natural layout, bias along free dim via replicated tile
            wr = w_sb["v"]
            for m in range(NT):
                pv = ps_proj.tile([P, QS], F32, tag="proj")
                for ki in range(DC):
                    nc.tensor.matmul(
                        pv[:], xT[:, ki, ts(m, P)], wr[:, ki, :],
                        start=(ki == 0), stop=(ki == DC - 1),
                    )
                nc.vector.tensor_add(out=V[:, m, :], in0=pv[:], in1=bv_rep[:])

        # ---- phase 3: attention, one 512-wide q strip at a time ----
        with tc.tile_pool(name="ps_st", bufs=2, space="PSUM") as ps_st, \
             tc.tile_pool(name="ps_sums", bufs=1, space="PSUM") as ps_sums, \
             tc.tile_pool(name="ps_ot", bufs=4, space="PSUM") as ps_ot, \
             tc.tile_pool(name="ps_fin", bufs=1, space="PSUM") as ps_fin:
            for s in range(NS):
                sums_ps = ps_sums.tile([P, QS], F32, tag="sums")
                ot_ps = [ps_ot.tile([P, QS], F32, tag="ot", name=f"ot_{s}_{c}")
                         for c in range(DC)]
                for kt in range(NT):
                    st = ps_st.tile([P, QS], F32, tag="st")
                    for c in range(DC):
                        nc.tensor.matmul(
                            st[:], KT[:, c, ts(kt, P)], QT[:, c, ts(s, QS)],
                            start=(c == 0), stop=(c == DC - 1),
                        )
                    e = epool.tile([P, QS], F32R, tag="e")
                    nc.scalar.activation(e[:], st[:], AF.Exp, scale=SOFTMAX_SCALE)
                    nc.tensor.matmul(
                        sums_ps[:], ones_r[:], e[:],
                        start=(kt == 0), stop=(kt == NT - 1),
                        skip_group_check=True,
                    )
                    for c in range(DC):
                        nc.tensor.matmul(
                            ot_ps[c][:], V[:, kt, ts(c, P)], e[:],
                            start=(kt == 0), stop=(kt == NT - 1),
                            skip_group_check=True,
                        )
                r = rpool.tile([P, QS], F32, tag="r")
                nc.vector.reciprocal(r[:], sums_ps[:])
                ot_sb = otpool.tile([P, DC, QS], F32, tag="ot_sb")
                for c in range(DC):
                    nc.vector.tensor_mul(out=ot_sb[:, c, :], in0=ot_ps[c][:], in1=r[:])
                # ---- transpose back to natural layout + store ----
                for lt in range(NS):      # 4 seq tiles inside this strip
                    t = s * NS + lt
                    stage = io512.tile([P, D], F32, tag="io512")
                    for c in range(DC):
                        tp2 = ps_fin.tile([P, P], F32, tag="fin")
                        nc.tensor.transpose(tp2[:], ot_sb[:, c, ts(lt, P)], ident)
                        nc.scalar.copy(stage[:, ts(c, P)], tp2[:])
                    nc.sync.dma_start(out[ts(t, P), :], stage[:])

    nc.compile()
    return nc


_CACHE = {}


def _get_nc():
    if "nc" not in _CACHE:
        _CACHE["nc"] = _build()
    return _CACHE["nc"]


def kernel(x, Wq_w, Wq_b, Wk_w, Wk_b, Wv_w, Wv_b, _trace=False, _tmpdir=None):
    nc = _get_nc()
    x = np.ascontiguousarray(np.asarray(x, dtype=np.float32))
    args = {
        "wq": Wq_w, "bq": Wq_b,
        "wk": Wk_w, "bk": Wk_b,
        "wv": Wv_w, "bv": Wv_b,
    }
    args = {k: np.ascontiguousarray(np.asarray(v, dtype=np.float32))
            for k, v in args.items()}
    in_maps = [dict(args, x=x[i]) for i in range(B)]
    res = bass_utils.run_bass_kernel_spmd(
        nc, in_maps, core_ids=list(range(B)),
        trace=_trace, tmpdir=_tmpdir,
    )
    out = np.stack([r["out"] for r in res.results], axis=0)
    if _trace:
        kernel.last_results = res
    return out


if __name__ == "__main__":
    rng = np.random.default_rng(0)
    inputs = {
        "x": rng.standard_normal((B, N, D)).astype(np.float32),
        "Wq_w": (0.02 * rng.standard_normal((D, D))).astype(np.float32),
        "Wq_b": np.zeros(D, np.float32),
        "Wk_w": (0.02 * rng.standard_normal((D, D))).astype(np.float32),
        "Wk_b": np.zeros(D, np.float32),
        "Wv_w": (0.02 * rng.standard_normal((D, D))).astype(np.float32),
        "Wv_b": np.zeros(D, np.float32),
    }
    got = kernel(**inputs)
    print("out shape:", got.shape, got.dtype)



# revision 6
# speedup vs baseline: 1.0731x; 1.0731x over previous
"""Single-head attention (B=8, N=2048, D=512, fp32) on 8 TRN2 NeuronCores.

Sharding: data-parallel over batch — core i computes batch element i
end-to-end (weights replicated). Per-core pipeline, matmuls in float32r
(full-rate PE at moving-dim >= 256, ~1e-4 relative rounding):

  x [2048,512] --PE transpose--> xT [512,2048]   (D on partitions)
  QT = Wq^T-contract -> [512,2048],  KT likewise (D on partitions)
  V  = x @ Wv -> [2048,512]          (seq on partitions)
  per 512-wide q strip:
    for each 128-row k tile kt:
      S^T tile [k=128,q=512] = KT-chunk^T @ QT    (accum over D chunks)
      E = exp(S^T / sqrt(D))                      (ACT, fused scale)
      per 128-col q subtile qt:
        O[qt]    += E[:,qt]^T @ V[kt]             (PSUM accum, natural layout)
        dsum[qt] += E[:,qt]^T @ ones              (N=1 matmul, same bank)
    O[qt] *= 1/dsum[qt]  (per-partition scalar)  ; DMA out (no transpose)

DMA spread: x on the two HW DGE queues (sync/scalar), weights+biases on
software DGE (gpsimd) queues, outputs alternate sync/scalar.
"""

import numpy as np

import concourse.bass as bass
import concourse.tile as tile
from concourse import bacc, mybir
from concourse import bass_utils
from concourse.bass import ts
from concourse.masks import make_identity
from contextlib import ExitStack

B, N, D = 8, 2048, 512
P = 128
NT = N // P      # 16 seq tiles
DC = D // P      # 4 d chunks
QS = 512         # q-strip width (one PSUM bank of fp32)
NS = N // QS     # 4 strips
QT_PER = QS // P # 4 q subtiles per strip
SOFTMAX_SCALE = 1.0 / float(np.sqrt(D))

F32 = mybir.dt.float32
F32R = mybir.dt.float32r
AF = mybir.ActivationFunctionType


def _build():
    nc = bacc.Bacc("TRN2", target_bir_lowering=False, debug=False)

    x = nc.dram_tensor("x", [N, D], F32, kind="ExternalInput").ap()
    wq = nc.dram_tensor("wq", [D, D], F32, kind="ExternalInput").ap()
    bq = nc.dram_tensor("bq", [D], F32, kind="ExternalInput").ap()
    wk = nc.dram_tensor("wk", [D, D], F32, kind="ExternalInput").ap()
    bk = nc.dram_tensor("bk", [D], F32, kind="ExternalInput").ap()
    wv = nc.dram_tensor("wv", [D, D], F32, kind="ExternalInput").ap()
    bv = nc.dram_tensor("bv", [D], F32, kind="ExternalInput").ap()
    out = nc.dram_tensor("out", [N, D], F32, kind="ExternalOutput").ap()

    with ExitStack() as ctx:
        tc = ctx.enter_context(tile.TileContext(nc))

        const = ctx.enter_context(tc.tile_pool(name="const", bufs=1))
        io512 = ctx.enter_context(tc.tile_pool(name="io512", bufs=6))
        wpool = ctx.enter_context(tc.tile_pool(name="wpool", bufs=2))
        wstage = ctx.enter_context(tc.tile_pool(name="wstage", bufs=2))
        big = ctx.enter_context(tc.tile_pool(name="big", bufs=1))
        epool = ctx.enter_context(tc.tile_pool(name="epool", bufs=3))
        opool = ctx.enter_context(tc.tile_pool(name="opool", bufs=3))
        rpool = ctx.enter_context(tc.tile_pool(name="rpool", bufs=2))

        # constants
        ident = const.tile([P, P], F32)
        make_identity(nc, ident)
        ones_f = const.tile([P, 2], F32)
        nc.vector.memset(ones_f, 1.0)
        ones_col = const.tile([P, 2], F32R)
        nc.vector.tensor_copy(out=ones_col[:], in_=ones_f[:])

        # biases via software DGE (keep HW queues free for x)
        bq_sb = const.tile([P, DC], F32)
        nc.gpsimd.dma_start(bq_sb[:], bq.rearrange("(c p) -> p c", p=P))
        bk_sb = const.tile([P, DC], F32)
        nc.gpsimd.dma_start(bk_sb[:], bk.rearrange("(c p) -> p c", p=P))
        bv_rep = const.tile([P, D], F32)
        nc.gpsimd.dma_start(bv_rep[:], bv[None, :].to_broadcast((P, D)))

        # weights: [ki, ko, dout] fp32r (round via DVE copy).
        # wq rides the software DGE so it lands while the HW queues carry x;
        # wk/wv enter the HW queues right after x tiles 0/1 (emitted below).
        w_sb = {}

        def _load_weight(name, wap, eng):
            wst = wstage.tile([P, DC, D], F32, tag="wstage")
            eng.dma_start(wst[:], wap.rearrange("(ko ki) d -> ki ko d", ki=P))
            wr = wpool.tile([P, DC, D], F32R, tag="w")
            nc.vector.tensor_copy(out=wr[:], in_=wst[:])
            w_sb[name] = wr

        _load_weight("q", wq, nc.gpsimd)

        # big persistent tensors
        xT = big.tile([P, DC, N], F32R)    # x^T: d on partitions
        QT = big.tile([P, DC, N], F32R)
        KT = big.tile([P, DC, N], F32R)
        V = big.tile([P, NT, D], F32R)     # natural: seq on partitions

        # ---- phase 1: load x tiles + PE-transpose into xT ----
        with tc.tile_pool(name="ps_tr", bufs=2, space="PSUM") as ps_tr, \
             tc.tile_pool(name="ps_proj", bufs=3, space="PSUM") as ps_proj:
            for t in range(NT):
                x_t = io512.tile([P, D], F32, tag="io512")
                eng = nc.sync if (t % 2 == 0) else nc.scalar
                eng.dma_start(x_t[:], x[ts(t, P), :])
                if t == 1:
                    # wk/wv queue behind x0/x1 on the two HW DGE queues
                    _load_weight("k", wk, nc.sync)
                    _load_weight("v", wv, nc.scalar)
                for c in range(DC):
                    tp = ps_tr.tile([P, P], F32, tag="tr")
                    nc.tensor.transpose(tp[:], x_t[:, ts(c, P)], ident)
                    nc.vector.tensor_copy(out=xT[:, c, ts(t, P)], in_=tp[:])

            # ---- phase 2: projections ----
            # QT/KT: [dout-chunk co on partitions, q on free]
            for name, dst, b_sb in (("q", QT, bq_sb), ("k", KT, bk_sb)):
                wr = w_sb[name]
                for co in range(DC):
                    for s in range(NS):
                        pq = ps_proj.tile([P, QS], F32, tag="proj")
                        for ki in range(DC):
                            nc.tensor.matmul(
                                pq[:], wr[:, ki, ts(co, P)], xT[:, ki, ts(s, QS)],
                                start=(ki == 0), stop=(ki == DC - 1),
                            )
                        # bias add (per-partition) + round to fp32r on ACT
                        nc.scalar.activation(
                            dst[:, co, ts(s, QS)], pq[:], AF.Identity,
                            bias=b_sb[:, co:co + 1],
                        )
            # V: natural layout, bias along free dim via replicated tile
            wr = w_sb["v"]
            for m in range(NT):
                pv = ps_proj.tile([P, QS], F32, tag="proj")
                for ki in range(DC):
                    nc.tensor.matmul(
                        pv[:], xT[:, ki, ts(m, P)], wr[:, ki, :],
                        start=(ki == 0), stop=(ki == DC - 1),
                    )
                nc.vector.tensor_add(out=V[:, m, :], in0=pv[:], in1=bv_rep[:])

        # ---- phase 3: attention, natural-layout O accumulation ----
        with tc.tile_pool(name="ps_st", bufs=2, space="PSUM") as ps_st, \
             tc.tile_pool(name="ps_o", bufs=4, space="PSUM") as ps_o, \
             tc.tile_pool(name="ps_ds", bufs=2, space="PSUM") as ps_ds:
            for s in range(NS):
                o_ps = [ps_o.tile([P, QS], F32, tag="o", name=f"o_{s}_{qt}")
                        for qt in range(QT_PER)]
                dsum = ps_ds.tile([P, 2 * QT_PER], F32, tag="ds")
                # zero data; all dsum matmuls accumulate with start=False so
                # correctness doesn't depend on has_written clear granularity
                nc.vector.memset(dsum, 0.0)
                for kt in range(NT):
                    st = ps_st.tile([P, QS], F32, tag="st")
                    for c in range(DC):
                        nc.tensor.matmul(
                            st[:], KT[:, c, ts(kt, P)], QT[:, c, ts(s, QS)],
                            start=(c == 0), stop=(c == DC - 1),
                        )
                    e = epool.tile([P, QS], F32R, tag="e")
                    nc.scalar.activation(e[:], st[:], AF.Exp, scale=SOFTMAX_SCALE)
                    for qt in range(QT_PER):
                        nc.tensor.matmul(
                            o_ps[qt][:], e[:, ts(qt, P)], V[:, kt, :],
                            start=(kt == 0), stop=(kt == NT - 1),
                            skip_group_check=True,
                        )
                        nc.tensor.matmul(
                            dsum[:, ts(qt, 2)], e[:, ts(qt, P)], ones_col[:],
                            start=False, stop=(kt == NT - 1),
                            skip_group_check=True,
                        )
                r = rpool.tile([P, 2 * QT_PER], F32, tag="r")
                nc.vector.reciprocal(r[:], dsum[:])
                for qt in range(QT_PER):
                    ob = opool.tile([P, QS], F32, tag="ob")
                    nc.vector.tensor_scalar_mul(ob[:], o_ps[qt][:], r[:, 2 * qt:2 * qt + 1])
                    eng = nc.sync if (qt % 2 == 0) else nc.scalar
                    eng.dma_start(out[ts(s * QT_PER + qt, P), :], ob[:])

    nc.compile()
    return nc


_CACHE = {}


def _get_nc():
    if "nc" not in _CACHE:
        _CACHE["nc"] = _build()
    return _CACHE["nc"]


def kernel(x, Wq_w, Wq_b, Wk_w, Wk_b, Wv_w, Wv_b, _trace=False, _tmpdir=None):
    nc = _get_nc()
    x = np.ascontiguousarray(np.asarray(x, dtype=np.float32))
    args = {
        "wq": Wq_w, "bq": Wq_b,
        "wk": Wk_w, "bk": Wk_b,
        "wv": Wv_w, "bv": Wv_b,
    }
    args = {k: np.ascontiguousarray(np.asarray(v, dtype=np.float32))
            for k, v in args.items()}
    in_maps = [dict(args, x=x[i]) for i in range(B)]
    res = bass_utils.run_bass_kernel_spmd(
        nc, in_maps, core_ids=list(range(B)),
        trace=_trace, tmpdir=_tmpdir,
    )
    out = np.stack([r["out"] for r in res.results], axis=0)
    if _trace:
        kernel.last_results = res
    return out


if __name__ == "__main__":
    rng = np.random.default_rng(0)
    inputs = {
        "x": rng.standard_normal((B, N, D)).astype(np.float32),
        "Wq_w": (0.02 * rng.standard_normal((D, D))).astype(np.float32),
        "Wq_b": np.zeros(D, np.float32),
        "Wk_w": (0.02 * rng.standard_normal((D, D))).astype(np.float32),
        "Wk_b": np.zeros(D, np.float32),
        "Wv_w": (0.02 * rng.standard_normal((D, D))).astype(np.float32),
        "Wv_b": np.zeros(D, np.float32),
    }
    got = kernel(**inputs)
    print("out shape:", got.shape, got.dtype)


# revision 7
# speedup vs baseline: 1.1988x; 1.1171x over previous
"""Single-head attention (B=8, N=2048, D=512, fp32) on 8 TRN2 NeuronCores.

Sharding: data-parallel over batch — core i computes batch element i
end-to-end (weights replicated). Per-core pipeline, matmuls in float32r
(full-rate PE at moving-dim >= 256, ~1e-4 relative rounding):

  x [2048,512] --PE transpose--> xT [512,2048]   (D on partitions)
  QT = Wq^T-contract -> [512,2048],  KT likewise (D on partitions)
  V  = x @ Wv -> [2048,512]          (seq on partitions)
  per 512-wide q strip:
    for each 128-row k tile kt:
      S^T tile [k=128,q=512] = KT-chunk^T @ QT    (accum over D chunks)
      E = exp(S^T / sqrt(D))                      (ACT, fused scale)
      per 128-col q subtile qt:
        O[qt]    += E[:,qt]^T @ V[kt]             (PSUM accum, natural layout)
        dsum[qt] += E[:,qt]^T @ ones              (N=1 matmul, same bank)
    O[qt] *= 1/dsum[qt]  (per-partition scalar)  ; DMA out (no transpose)

DMA spread: x on the two HW DGE queues (sync/scalar), weights+biases on
software DGE (gpsimd) queues, outputs alternate sync/scalar.
"""

import numpy as np

import concourse.bass as bass
import concourse.tile as tile
from concourse import bacc, mybir
from concourse import bass_utils
from concourse.bass import ts
from concourse.masks import make_identity
from contextlib import ExitStack

B, N, D = 8, 2048, 512
P = 128
NT = N // P      # 16 seq tiles
DC = D // P      # 4 d chunks
QS = 512         # q-strip width (one PSUM bank of fp32)
NS = N // QS     # 4 strips
QT_PER = QS // P # 4 q subtiles per strip
SOFTMAX_SCALE = 1.0 / float(np.sqrt(D))

F32 = mybir.dt.float32
F32R = mybir.dt.float32r
BF16 = mybir.dt.bfloat16
AF = mybir.ActivationFunctionType


def _build():
    nc = bacc.Bacc("TRN2", target_bir_lowering=False, debug=False)

    x = nc.dram_tensor("x", [N, D], F32, kind="ExternalInput").ap()
    wq = nc.dram_tensor("wq", [D, D], F32, kind="ExternalInput").ap()
    bq = nc.dram_tensor("bq", [D], F32, kind="ExternalInput").ap()
    wk = nc.dram_tensor("wk", [D, D], F32, kind="ExternalInput").ap()
    bk = nc.dram_tensor("bk", [D], F32, kind="ExternalInput").ap()
    wv = nc.dram_tensor("wv", [D, D], F32, kind="ExternalInput").ap()
    bv = nc.dram_tensor("bv", [D], F32, kind="ExternalInput").ap()
    out = nc.dram_tensor("out", [N, D], F32, kind="ExternalOutput").ap()

    with ExitStack() as ctx:
        tc = ctx.enter_context(tile.TileContext(nc))

        const = ctx.enter_context(tc.tile_pool(name="const", bufs=1))
        io512 = ctx.enter_context(tc.tile_pool(name="io512", bufs=6))
        wpool = ctx.enter_context(tc.tile_pool(name="wpool", bufs=2))
        wstage = ctx.enter_context(tc.tile_pool(name="wstage", bufs=2))
        big = ctx.enter_context(tc.tile_pool(name="big", bufs=1))
        epool = ctx.enter_context(tc.tile_pool(name="epool", bufs=3))
        opool = ctx.enter_context(tc.tile_pool(name="opool", bufs=3))
        rpool = ctx.enter_context(tc.tile_pool(name="rpool", bufs=2))

        # constants
        ident = const.tile([P, P], F32)
        make_identity(nc, ident)
        ones_col = const.tile([P, 1], BF16)
        nc.vector.memset(ones_col, 1.0)

        # biases via software DGE (keep HW queues free for x)
        bq_sb = const.tile([P, DC], F32)
        nc.gpsimd.dma_start(bq_sb[:], bq.rearrange("(c p) -> p c", p=P))
        bk_sb = const.tile([P, DC], F32)
        nc.gpsimd.dma_start(bk_sb[:], bk.rearrange("(c p) -> p c", p=P))
        bv_rep = const.tile([P, D], F32)
        nc.gpsimd.dma_start(bv_rep[:], bv[None, :].to_broadcast((P, D)))

        # weights: [ki, ko, dout] fp32r (round via DVE copy).
        # wq rides the software DGE so it lands while the HW queues carry x;
        # wk/wv enter the HW queues right after x tiles 0/1 (emitted below).
        w_sb = {}

        def _load_weight(name, wap, eng):
            wst = wstage.tile([P, DC, D], F32, tag="wstage")
            eng.dma_start(wst[:], wap.rearrange("(ko ki) d -> ki ko d", ki=P))
            wr = wpool.tile([P, DC, D], F32R, tag="w")
            nc.vector.tensor_copy(out=wr[:], in_=wst[:])
            w_sb[name] = wr

        _load_weight("q", wq, nc.gpsimd)

        # big persistent tensors
        xT = big.tile([P, DC, N], F32R)    # x^T: d on partitions
        QT = big.tile([P, DC, N], F32R)
        KT = big.tile([P, DC, N], F32R)
        V = big.tile([P, NT, D], BF16)     # natural: seq on partitions (bf16)

        # ---- phase 1: load x tiles + PE-transpose into xT ----
        with tc.tile_pool(name="ps_tr", bufs=2, space="PSUM") as ps_tr, \
             tc.tile_pool(name="ps_proj", bufs=3, space="PSUM") as ps_proj:
            for t in range(NT):
                x_t = io512.tile([P, D], F32, tag="io512")
                eng = nc.sync if (t % 2 == 0) else nc.scalar
                eng.dma_start(x_t[:], x[ts(t, P), :])
                if t == 1:
                    # wk/wv queue behind x0/x1 on the two HW DGE queues
                    _load_weight("k", wk, nc.sync)
                    _load_weight("v", wv, nc.scalar)
                for c in range(DC):
                    tp = ps_tr.tile([P, P], F32, tag="tr")
                    nc.tensor.transpose(tp[:], x_t[:, ts(c, P)], ident)
                    nc.vector.tensor_copy(out=xT[:, c, ts(t, P)], in_=tp[:])

            # ---- phase 2: projections ----
            # QT/KT: [dout-chunk co on partitions, q on free]
            for name, dst, b_sb in (("q", QT, bq_sb), ("k", KT, bk_sb)):
                wr = w_sb[name]
                for co in range(DC):
                    for s in range(NS):
                        pq = ps_proj.tile([P, QS], F32, tag="proj")
                        for ki in range(DC):
                            nc.tensor.matmul(
                                pq[:], wr[:, ki, ts(co, P)], xT[:, ki, ts(s, QS)],
                                start=(ki == 0), stop=(ki == DC - 1),
                            )
                        # bias add (per-partition) + round to fp32r on ACT
                        nc.scalar.activation(
                            dst[:, co, ts(s, QS)], pq[:], AF.Identity,
                            bias=b_sb[:, co:co + 1],
                        )
            # V: natural layout, bias along free dim via replicated tile
            wr = w_sb["v"]
            for m in range(NT):
                pv = ps_proj.tile([P, QS], F32, tag="proj")
                for ki in range(DC):
                    nc.tensor.matmul(
                        pv[:], xT[:, ki, ts(m, P)], wr[:, ki, :],
                        start=(ki == 0), stop=(ki == DC - 1),
                    )
                nc.vector.tensor_add(out=V[:, m, :], in0=pv[:], in1=bv_rep[:])

        # ---- phase 3: attention, natural-layout O accumulation ----
        with tc.tile_pool(name="ps_st", bufs=2, space="PSUM") as ps_st, \
             tc.tile_pool(name="ps_o", bufs=4, space="PSUM") as ps_o, \
             tc.tile_pool(name="ps_ds", bufs=2, space="PSUM") as ps_ds:
            for s in range(NS):
                o_ps = [ps_o.tile([P, QS], F32, tag="o", name=f"o_{s}_{qt}")
                        for qt in range(QT_PER)]
                dsum = ps_ds.tile([P, QT_PER], F32, tag="ds")
                # zero data; all dsum matmuls accumulate with start=False so
                # correctness doesn't depend on has_written clear granularity
                nc.vector.memset(dsum, 0.0)
                for kt in range(NT):
                    st = ps_st.tile([P, QS], F32, tag="st")
                    for c in range(DC):
                        nc.tensor.matmul(
                            st[:], KT[:, c, ts(kt, P)], QT[:, c, ts(s, QS)],
                            start=(c == 0), stop=(c == DC - 1),
                        )
                    e = epool.tile([P, QS], BF16, tag="e")
                    nc.scalar.activation(e[:], st[:], AF.Exp, scale=SOFTMAX_SCALE)
                    for qt in range(QT_PER):
                        nc.tensor.matmul(
                            o_ps[qt][:], e[:, ts(qt, P)], V[:, kt, :],
                            start=(kt == 0), stop=(kt == NT - 1),
                            skip_group_check=True,
                        )
                        nc.tensor.matmul(
                            dsum[:, qt:qt + 1], e[:, ts(qt, P)], ones_col[:],
                            start=False, stop=(kt == NT - 1),
                            skip_group_check=True,
                        )
                r = rpool.tile([P, QT_PER], F32, tag="r")
                nc.vector.reciprocal(r[:], dsum[:])
                for qt in range(QT_PER):
                    ob = opool.tile([P, QS], F32, tag="ob")
                    nc.vector.tensor_scalar_mul(ob[:], o_ps[qt][:], r[:, qt:qt + 1])
                    eng = nc.sync if (qt % 2 == 0) else nc.scalar
                    eng.dma_start(out[ts(s * QT_PER + qt, P), :], ob[:])

    nc.compile()
    return nc


_CACHE = {}


def _get_nc():
    if "nc" not in _CACHE:
        _CACHE["nc"] = _build()
    return _CACHE["nc"]


def kernel(x, Wq_w, Wq_b, Wk_w, Wk_b, Wv_w, Wv_b, _trace=False, _tmpdir=None):
    nc = _get_nc()
    x = np.ascontiguousarray(np.asarray(x, dtype=np.float32))
    args = {
        "wq": Wq_w, "bq": Wq_b,
        "wk": Wk_w, "bk": Wk_b,
        "wv": Wv_w, "bv": Wv_b,
    }
    args = {k: np.ascontiguousarray(np.asarray(v, dtype=np.float32))
            for k, v in args.items()}
    in_maps = [dict(args, x=x[i]) for i in range(B)]
    res = bass_utils.run_bass_kernel_spmd(
        nc, in_maps, core_ids=list(range(B)),
        trace=_trace, tmpdir=_tmpdir,
    )
    out = np.stack([r["out"] for r in res.results], axis=0)
    if _trace:
        kernel.last_results = res
    return out


if __name__ == "__main__":
    rng = np.random.default_rng(0)
    inputs = {
        "x": rng.standard_normal((B, N, D)).astype(np.float32),
        "Wq_w": (0.02 * rng.standard_normal((D, D))).astype(np.float32),
        "Wq_b": np.zeros(D, np.float32),
        "Wk_w": (0.02 * rng.standard_normal((D, D))).astype(np.float32),
        "Wk_b": np.zeros(D, np.float32),
        "Wv_w": (0.02 * rng.standard_normal((D, D))).astype(np.float32),
        "Wv_b": np.zeros(D, np.float32),
    }
    got = kernel(**inputs)
    print("out shape:", got.shape, got.dtype)
